# revision 2
# baseline (speedup 1.0000x reference)
"""AtomDistances Trainium2 kernel (nn_AtomDistances_41051297415622).

kernel(positions, neighbors, neighbor_mask) -> distances [64, 512, 256] f32.

Data-parallel over 8 NeuronCores: core c handles batches [8c, 8c+8).

Per-core program (Bass/Tile, SPMD single NEFF):
  1. Per batch, build the full 512x512 distance matrix D on PE/ACT via the
     Gram identity  D[r,n] = sqrt(max(n2[r] + n2[n] - 2 p_r.p_n, 0)),
     stored per 128-row tile as a gather table [128, 513] whose entry 0 is
     0.0 (the target for masked and self edges), entries 1..512 = D row.
  2. One DVE pass computes per-edge table indices
        idx' = mask * (idx + 1) * (idx != row)
     reading the low 16 bits of the int64/int32 neighbor words with a
     strided access pattern (no separate conversion pass).
  3. 16 SBUF->SBUF DMAs per tile pre-shuffle the indices across the
     16-partition GpSimd core groups so that each core's ap_gather writes
     row r's 256 gathered distances contiguously at G[r, (r%16)*256 ..].
  4. GpSimd ap_gather (one instruction per 128-row tile, 4096 indices per
     core) performs the data-dependent lookup from the per-row tables.
  5. 16 partition-strided DMAs per tile write the gathered rows straight
     to the output in DRAM.
"""

import numpy as np

_CACHE = {}


def _build(B_PER, IDXW):
    import concourse.bacc as bacc
    import concourse.mybir as mybir
    from concourse.tile import TileContext

    F32 = mybir.dt.float32
    I16 = mybir.dt.int16
    I32 = mybir.dt.int32
    N, NB = 512, 256
    TPB = N // 128

    nc = bacc.Bacc()
    pos = nc.dram_tensor("positions", [B_PER, N, 3], F32, kind="ExternalInput")
    nbr16 = nc.dram_tensor("nbr16", [B_PER, N, NB * IDXW], I16, kind="ExternalInput")
    mask = nc.dram_tensor("mask", [B_PER, N, NB], I32, kind="ExternalInput")
    rowid = nc.dram_tensor("rowid", [N, 1], F32, kind="ExternalInput")
    out = nc.dram_tensor("out", [B_PER, N, NB], F32, kind="ExternalOutput")

    with TileContext(nc) as tc:
        with (
            tc.tile_pool(name="consts", bufs=1) as cpool,
            tc.tile_pool(name="ptab", bufs=2) as ppool,
            tc.tile_pool(name="dtab", bufs=2 * TPB) as dpool,
            tc.tile_pool(name="work", bufs=3) as wpool,
            tc.tile_pool(name="gbuf", bufs=2) as gpool,
            tc.tile_pool(name="psum", bufs=2, space="PSUM") as pspool,
            tc.tile_pool(name="psum1", bufs=2, space="PSUM") as ps1pool,
        ):
            ones3 = cpool.tile([3, 1], F32, tag="ones3")
            nc.vector.memset(ones3[:], 1.0)
            ones1 = cpool.tile([1, 128], F32, tag="ones1")
            nc.vector.memset(ones1[:], 1.0)

            for b in range(B_PER):
                # ---- per-batch distance tables ----
                PT = ppool.tile([3, N], F32, tag="PT")
                nc.sync.dma_start(out=PT[:], in_=pos[b].rearrange("n c -> c n"))
                PTm2 = ppool.tile([3, N], F32, tag="PTm2")
                nc.vector.tensor_scalar_mul(PTm2[:], PT[:], -2.0)
                sqPT = ppool.tile([3, N], F32, tag="sqPT")
                nc.vector.tensor_mul(sqPT[:], PT[:], PT[:])
                n2ps = ps1pool.tile([1, N], F32, tag="n2ps")
                nc.tensor.matmul(n2ps[:], ones3[:], sqPT[:], start=True, stop=True)
                n2row = ppool.tile([1, N], F32, tag="n2row")
                nc.vector.tensor_copy(n2row[:], n2ps[:])

                Ptiles = ppool.tile([128, 3 * TPB], F32, tag="Ptiles")
                nc.sync.dma_start(
                    out=Ptiles[:].rearrange("p (t c) -> p t c", c=3),
                    in_=pos[b].rearrange("(t p) c -> p t c", p=128),
                )
                sq = ppool.tile([128, 3 * TPB], F32, tag="sq")
                nc.vector.tensor_mul(sq[:], Ptiles[:], Ptiles[:])
                n2a = ppool.tile([128, TPB], F32, tag="n2a")
                nc.vector.tensor_add(
                    n2a[:], sq[:, 0 : 3 * TPB : 3], sq[:, 1 : 3 * TPB : 3]
                )
                n2t = ppool.tile([128, TPB], F32, tag="n2t")
                nc.vector.tensor_add(n2t[:], n2a[:], sq[:, 2 : 3 * TPB : 3])

                dts = []
                for t in range(TPB):
                    Gp = pspool.tile([128, N], F32, tag="gram")
                    nc.tensor.matmul(
                        Gp[:],
                        PTm2[:, t * 128 : (t + 1) * 128],
                        PT[:],
                        start=True,
                        stop=False,
                    )
                    nc.tensor.matmul(Gp[:], ones1[:], n2row[:], start=False, stop=True)
                    U = wpool.tile([128, N], F32, tag="U")
                    nc.vector.tensor_scalar(
                        out=U[:],
                        in0=Gp[:],
                        scalar1=n2t[:, t : t + 1],
                        scalar2=0.0,
                        op0=mybir.AluOpType.add,
                        op1=mybir.AluOpType.max,
                    )
                    D = dpool.tile([128, N + 1], F32, tag="D")
                    nc.vector.memset(D[:, 0:1], 0.0)
                    nc.scalar.activation(
                        D[:, 1 : N + 1], U[:], mybir.ActivationFunctionType.Sqrt
                    )
                    dts.append(D)

                # ---- edge processing ----
                for t in range(TPB):
                    rs = slice(t * 128, (t + 1) * 128)
                    nb = wpool.tile([128, NB * IDXW], I16, tag="nb")
                    nc.sync.dma_start(out=nb[:], in_=nbr16[b, rs])
                    mk = wpool.tile([128, NB], I32, tag="mk")
                    nc.sync.dma_start(out=mk[:], in_=mask[b, rs])
                    # L2[row, 16*beta + alpha] = mask*(idx+1)*(idx != row)
                    # for j = 16*alpha + beta; masked/self edges point at 0.0
                    nbv = nb[:, 0 : NB * IDXW : IDXW].rearrange(
                        "p (a b) -> p b a", a=16, b=16
                    )
                    rid = wpool.tile([128, 1], F32, tag="rid")
                    nc.sync.dma_start(out=rid[:], in_=rowid[rs])
                    NE = wpool.tile([128, NB], F32, tag="NE")
                    nc.vector.tensor_scalar(
                        out=NE[:],
                        in0=nbv,
                        scalar1=rid[:],
                        scalar2=None,
                        op0=mybir.AluOpType.not_equal,
                    )
                    L1 = wpool.tile([128, NB], I16, tag="L1")
                    nc.vector.scalar_tensor_tensor(
                        out=L1[:],
                        in0=nbv,
                        scalar=1.0,
                        in1=mk[:].rearrange("p (a b) -> p b a", a=16, b=16),
                        op0=mybir.AluOpType.add,
                        op1=mybir.AluOpType.mult,
                    )
                    L2 = wpool.tile([128, NB], I16, tag="L2")
                    nc.vector.tensor_mul(L2[:], L1[:], NE[:])
                    # W[16g+beta, 16p+alpha] = L2[16g+p, 16beta+alpha]
                    W = wpool.tile([128, NB], I16, tag="W")
                    for p in range(16):
                        nc.sync.dma_start(
                            out=W[:, 16 * p : 16 * p + 16], in_=L2[p:128:16, :]
                        )
                    G = gpool.tile([128, 16 * NB], F32, tag="G")
                    nc.gpsimd.ap_gather(
                        G[:].rearrange("p (i d) -> p i d", d=1),
                        dts[t][:].rearrange("p (n d) -> p n d", d=1),
                        W[:],
                        channels=128,
                        num_elems=N + 1,
                        d=1,
                        num_idxs=16 * NB,
                    )
                    # G[16g+p, 256p + j] = dist(row 16g+p, j): write out directly
                    for p in range(16):
                        nc.scalar.dma_start(
                            out=out[b, t * 128 + p : (t + 1) * 128 : 16, :],
                            in_=G[p:128:16, NB * p : NB * (p + 1)],
                        )

    nc.compile()
    return nc


def _get_runner(IDXW, B_PER=8, n_cores=8):
    key = (IDXW, B_PER, n_cores)
    if key in _CACHE:
        return _CACHE[key]

    import jax
    import concourse.mybir as mybir
    from concourse.bass2jax import (
        _bass_exec_p,
        install_neuronx_cc_hook,
        partition_id_tensor,
    )
    from jax.sharding import Mesh, PartitionSpec
    from jax.experimental.shard_map import shard_map

    nc = _build(B_PER, IDXW)

    install_neuronx_cc_hook()
    partition_name = nc.partition_id_tensor.name if nc.partition_id_tensor else None
    in_names, out_names, out_avals = [], [], []
    for alloc in nc.m.functions[0].allocations:
        if not isinstance(alloc, mybir.MemoryLocationSet):
            continue
        name = alloc.memorylocations[0].name
        if alloc.kind == "ExternalInput":
            if name != partition_name:
                in_names.append(name)
        elif alloc.kind == "ExternalOutput":
            out_names.append(name)
            out_avals.append(
                jax.core.ShapedArray(tuple(alloc.tensor_shape), mybir.dt.np(alloc.dtype))
            )
    n_params = len(in_names)
    all_in_names = list(in_names) + out_names
    if partition_name is not None:
        all_in_names.append(partition_name)

    def _body(*args):
        operands = list(args)
        if partition_name is not None:
            operands.append(partition_id_tensor())
        return tuple(
            _bass_exec_p.bind(
                *operands,
                out_avals=tuple(out_avals),
                in_names=tuple(all_in_names),
                out_names=tuple(out_names),
                lowering_input_output_aliases=(),
                sim_require_finite=True,
                sim_require_nnan=True,
                nc=nc,
            )
        )

    devices = jax.devices()[:n_cores]
    mesh = Mesh(np.asarray(devices), ("core",))
    n_outs = len(out_names)
    in_specs = (PartitionSpec("core"),) * (n_params + n_outs)
    out_specs = (PartitionSpec("core"),) * n_outs
    donate = tuple(range(n_params, n_params + n_outs))
    sharded = jax.jit(
        shard_map(_body, mesh=mesh, in_specs=in_specs, out_specs=out_specs, check_rep=False),
        donate_argnums=donate,
        keep_unused=True,
    )

    def run(in_maps):
        concat_in = [
            np.concatenate([np.asarray(in_maps[c][n]) for c in range(n_cores)], axis=0)
            for n in in_names
        ]
        zeros = [np.zeros((n_cores * a.shape[0], *a.shape[1:]), a.dtype) for a in out_avals]
        outs = sharded(*concat_in, *zeros)
        outs = [np.asarray(o) for o in outs]
        return [
            {n: outs[i].reshape(n_cores, *out_avals[i].shape)[c] for i, n in enumerate(out_names)}
            for c in range(n_cores)
        ]

    _CACHE[key] = run
    return run


def _make_in_maps(positions, neighbors, neighbor_mask, B_PER=8, n_cores=8):
    pos = np.ascontiguousarray(np.asarray(positions), dtype=np.float32)
    nbr = np.ascontiguousarray(np.asarray(neighbors))
    if nbr.dtype not in (np.dtype(np.int64), np.dtype(np.int32)):
        nbr = nbr.astype(np.int64)
    msk = np.asarray(neighbor_mask)
    if msk.dtype != np.int32:
        msk = msk.astype(np.int32)
    msk = np.ascontiguousarray(msk)
    IDXW = nbr.dtype.itemsize // 2
    rowid_arr = np.arange(512, dtype=np.float32).reshape(512, 1)
    in_maps = []
    for c in range(n_cores):
        s = slice(c * B_PER, (c + 1) * B_PER)
        in_maps.append(
            {
                "positions": pos[s],
                "nbr16": np.ascontiguousarray(nbr[s]).view(np.int16).reshape(
                    B_PER, 512, 256 * IDXW
                ),
                "mask": msk[s],
                "rowid": rowid_arr,
            }
        )
    return in_maps, IDXW


def kernel(positions, neighbors, neighbor_mask):
    in_maps, IDXW = _make_in_maps(positions, neighbors, neighbor_mask)
    from concourse._compat import axon_active

    if axon_active():
        run = _get_runner(IDXW)
        res = run(in_maps)
    else:
        from concourse.bass_utils import run_bass_kernel_spmd

        key = ("nc", IDXW)
        if key not in _CACHE:
            _CACHE[key] = _build(8, IDXW)
        res = run_bass_kernel_spmd(
            _CACHE[key], in_maps, core_ids=list(range(8))
        ).results
    return np.concatenate([r["out"] for r in res], axis=0)
